# revision 19
# baseline (speedup 1.0000x reference)
"""MemoAttention Trainium2 kernel — 8-core SPMD (v2).

Reference computation (B=2, S=2048, D=1024, H=16, Dh=64):
    qp = q @ Wq; kp = k @ Wk; vp = v @ Wv          (per batch)
    scores = (qh @ kh^T) * (1/8) * 2*sigmoid(qh . sw_h)   per head
    attn   = softmax(scores); out = attn @ vh
    gate   = sigmoid(out @ Wg + gb); y = (out * gate) @ Wo

Sharding: core c owns heads {2c, 2c+1} for BOTH batches (head-parallel
attention in a transposed [feature, seq] layout).  Fine-grained per-(b,sc)
AllToAlls convert to sequence-parallel ownership (core c owns query block
512*sc + 64*c .. +63 of every (b, sc)) for the gate/out_proj stage, which
runs in two phases (one per batch).

Inputs are streamed in column panels of 512 sequence positions so the
projection matmuls chase the DMA (no dead ramp), and the PE is pre-warmed
with junk matmuls so the HAM clock gate opens before real work arrives.
Sigmoids are computed as (1 + tanh(x/2))/2 — tanh shares the exp table set,
so the Scalar engine never reloads activation tables.

All device compute in bf16 (fp32 PSUM accumulate).  Host does only layout
prep (transpose / slice / dtype cast) and the final unshard.
"""

import os
import sys

import numpy as np

sys.path.insert(0, "/opt/trn_rl_repo")

import ml_dtypes

import concourse.bacc as bacc
import concourse.bass as bass
import concourse.bass_utils as bass_utils
import concourse.tile as tile
from concourse import mybir

BF16 = ml_dtypes.bfloat16

D_MODEL = 1024
NHEAD = 16
HEAD_DIM = 64
B = 2
S = 2048
N_CORES = 8
GCOLS = 2 * HEAD_DIM        # 128 d_model columns per core (2 heads)
S_CHUNK = 512
N_SC = S // S_CHUNK         # 4
N_TT = S // 128             # 16 t-tiles
N_IC = D_MODEL // 128       # 8 contraction chunks
PAN = 512                   # panel width (seq positions per input panel)
N_PAN = S // PAN            # 4

FP32 = mybir.dt.float32
BF16_T = mybir.dt.bfloat16

TRACE = False
TRACE_KWARGS = {}
LAST_RESULTS = None
DEBUG = False

_CACHE = {}


def _ensure_ntff_hook():
    """The agent image's antenv lacks axon_hooks; synthesize it so
    run_bass_kernel_spmd(trace=True) can NTFF-profile via libaxon_pjrt."""
    import types

    try:
        from antenv import axon_hooks  # noqa: F401
        return
    except ImportError:
        pass
    import antenv

    mod = types.ModuleType("antenv.axon_hooks")
    _state = {"hook": None}
    mod.set_axon_ntff_profile_hook = lambda h: _state.__setitem__("hook", h)
    mod.get_axon_ntff_profile_hook = lambda: _state["hook"]
    sys.modules["antenv.axon_hooks"] = mod
    antenv.axon_hooks = mod
    try:
        from trn_agent_boot.trn_boot import _ntff_profile_via_ctypes

        hook = _ntff_profile_via_ctypes("/opt/axon/libaxon_pjrt.so")
        if hook is not None:
            mod.set_axon_ntff_profile_hook(hook)
    except Exception as e:  # pragma: no cover
        print(f"ntff hook setup failed ({e}); tracing disabled", file=sys.stderr)


def _build_nc():
    nc = bacc.Bacc(
        "TRN2",
        target_bir_lowering=False,
        debug=False,
        enable_asserts=True,
        num_devices=N_CORES,
    )

    # ---- I/O ----
    # panel-major transposed inputs: [b, part, panel, chunk*PAN]
    # element (b, p, pan, 512*i + s) = X[b, 512*pan + s, 128*i + p]
    qT = nc.dram_tensor("qT", [B, 128, N_PAN, N_IC * PAN], BF16_T,
                        kind="ExternalInput").ap()
    kT = nc.dram_tensor("kT", [B, 128, N_PAN, N_IC * PAN], BF16_T,
                        kind="ExternalInput").ap()
    vT = nc.dram_tensor("vT", [B, 128, N_PAN, N_IC * PAN], BF16_T,
                        kind="ExternalInput").ap()
    wq = nc.dram_tensor("wq", [128, N_IC * GCOLS], BF16_T, kind="ExternalInput").ap()
    wk = nc.dram_tensor("wk", [128, N_IC * GCOLS], BF16_T, kind="ExternalInput").ap()
    wv = nc.dram_tensor("wv", [128, N_IC * GCOLS], BF16_T, kind="ExternalInput").ap()
    swb = nc.dram_tensor("swb", [GCOLS, 2], BF16_T, kind="ExternalInput").ap()
    wg = nc.dram_tensor("wg", [128, N_IC * D_MODEL], BF16_T, kind="ExternalInput").ap()
    wo = nc.dram_tensor("wo", [128, N_IC * D_MODEL], BF16_T, kind="ExternalInput").ap()
    gb = nc.dram_tensor("gb", [128, 8], FP32, kind="ExternalInput").ap()
    yT = nc.dram_tensor("yT", [D_MODEL, 2 * 256], BF16_T, kind="ExternalOutput").ap()
    dbg = {}
    if DEBUG:
        for nm, shp, dt in [
            ("dbg_qhTs", [B, 128, S], BF16_T),
            ("dbg_kpTs", [B, 128, S], BF16_T),
            ("dbg_vp", [B, N_PAN, 128, 512], BF16_T),
            ("dbg_nrm", [B, N_SC, 128, S_CHUNK], BF16_T),
            ("dbg_oT", [B, N_IC, 128, 256], BF16_T),
        ]:
            dbg[nm] = nc.dram_tensor(nm, shp, dt, kind="ExternalOutput").ap()

    EXP = mybir.ActivationFunctionType.Exp
    TANH = mybir.ActivationFunctionType.Tanh
    MULT = mybir.AluOpType.mult
    ADD = mybir.AluOpType.add

    with tile.TileContext(nc) as tc:
        with tc.tile_pool(name="const", bufs=1) as cpool:
            ones_sb = cpool.tile([128, 1], BF16_T)
            nc.vector.memset(ones_sb[:], 1.0)
            junk_sb = cpool.tile([128, 512], BF16_T)
            nc.vector.memset(junk_sb[:], 0.125)
            wramp = cpool.tile([128, 32], FP32)

            wq_sb = cpool.tile([128, N_IC, GCOLS], BF16_T)
            nc.sync.dma_start(wq_sb[:], wq.rearrange("p (c n) -> p c n", c=N_IC))
            wk_sb = cpool.tile([128, N_IC, GCOLS], BF16_T)
            nc.sync.dma_start(wk_sb[:], wk.rearrange("p (c n) -> p c n", c=N_IC))
            wv_sb = cpool.tile([128, N_IC, GCOLS], BF16_T)
            nc.sync.dma_start(wv_sb[:], wv.rearrange("p (c n) -> p c n", c=N_IC))
            swb_sb = cpool.tile([128, 2], BF16_T)
            nc.sync.dma_start(swb_sb[:], swb)
            gb_sb = cpool.tile([128, 8], FP32)
            nc.sync.dma_start(gb_sb[:], gb)
            # stage-2 weights: DMAs emitted later (don't fight the input ramp)
            wg_sb = cpool.tile([128, N_IC, D_MODEL], BF16_T)
            wo_sb = cpool.tile([128, N_IC, D_MODEL], BF16_T)

            # persistent per-batch activation tensors
            qhTs = [cpool.tile([128, S], BF16_T, name=f"qhTs{b}") for b in range(B)]
            kpTs = [cpool.tile([128, S], BF16_T, name=f"kpTs{b}") for b in range(B)]
            vp_sb = [[cpool.tile([128, 512], BF16_T, name=f"vp{b}_{p}")
                      for p in range(N_PAN)] for b in range(B)]

            # ---------- pools ----------
            xpan_cm = tc.tile_pool(name="xpan", bufs=8)
            xpan = xpan_cm.__enter__()
            projp_cm = tc.tile_pool(name="projp", bufs=2, space="PSUM")
            projp = projp_cm.__enter__()
            attnp_cm = tc.tile_pool(name="attnp", bufs=8)
            attnp = attnp_cm.__enter__()
            scr_cm = tc.tile_pool(name="scr", bufs=3)
            scr = scr_cm.__enter__()
            nrm_cm = tc.tile_pool(name="nrm", bufs=2)
            nrm = nrm_cm.__enter__()
            pdram_cm = tc.tile_pool(name="pdram", bufs=4, space="DRAM")
            pdram = pdram_cm.__enter__()
            dram_cm = tc.tile_pool(name="dram", bufs=1, space="DRAM")
            dram = dram_cm.__enter__()
            s2sb_cm = tc.tile_pool(name="s2sb", bufs=1)
            s2sb = s2sb_cm.__enter__()
            # attention PSUM pools last — released mid-kernel (LIFO) to free
            # banks for the stage-2 tail accumulators
            spsp_cm = tc.tile_pool(name="spsp", bufs=2, space="PSUM")
            spsp = spsp_cm.__enter__()
            avp_cm = tc.tile_pool(name="avp", bufs=1, space="PSUM")
            avp = avp_cm.__enter__()
            denp_cm = tc.tile_pool(name="denp", bufs=1, space="PSUM")
            denp = denp_cm.__enter__()

            a2a_in = [dram.tile([N_CORES, 128, 256], BF16_T, name=f"a2ain{b}")
                      for b in range(B)]
            a2a_out = [dram.tile([N_CORES, 128, 256], BF16_T, name=f"a2aout{b}")
                       for b in range(B)]

            # ---- ramp: ACT table preload + PE warm-up + dummy A2A ----
            nc.scalar.activation(wramp[0:1, :], junk_sb[0:1, 0:32], EXP)
            wps = projp.tile([128, 512], FP32, tag="proj")
            for _ in range(10):
                nc.tensor.matmul(wps[0:1, :], lhsT=ones_sb[:], rhs=junk_sb[:],
                                 start=True, stop=True)
            dmy_sb = scr.tile([8, 16], BF16_T, tag="dmy")
            nc.vector.memset(dmy_sb[:], 1.0)
            dmy_in = dram.tile([N_CORES, 1, 16], BF16_T, name="dmy_in")
            dmy_out = dram.tile([N_CORES, 1, 16], BF16_T, name="dmy_out")
            nc.sync.dma_start(dmy_in.rearrange("a b c -> (a b) c"), dmy_sb[:])
            nc.gpsimd.collective_compute(
                "AllToAll", mybir.AluOpType.bypass,
                replica_groups=[list(range(N_CORES))],
                ins=[dmy_in.opt()], outs=[dmy_out.opt()])

            def emit_panel(b, kind, p):
                src = {"q": qT, "k": kT, "v": vT}[kind]
                pan = xpan.tile([128, N_IC * PAN], BF16_T, tag="xp",
                                name=f"pan_{kind}{b}_{p}")
                nc.sync.dma_start(pan[:], src[b, :, p, :])
                ssl = slice(PAN * p, PAN * (p + 1))
                if kind == "k":
                    ps = projp.tile([128, 512], FP32, tag="proj")
                    for i in range(N_IC):
                        nc.tensor.matmul(
                            ps[:], lhsT=wk_sb[:, i, :],
                            rhs=pan[:, PAN * i:PAN * (i + 1)],
                            start=(i == 0), stop=(i == N_IC - 1))
                    nc.vector.tensor_copy(kpTs[b][:, ssl], ps[:])
                elif kind == "q":
                    ps = projp.tile([128, 512], FP32, tag="proj")
                    for i in range(N_IC):
                        nc.tensor.matmul(
                            ps[:], lhsT=wq_sb[:, i, :],
                            rhs=pan[:, PAN * i:PAN * (i + 1)],
                            start=(i == 0), stop=(i == N_IC - 1))
                    qp8 = scr.tile([128, 512], BF16_T, tag="qp8")
                    nc.vector.tensor_scalar_mul(qp8[:], ps[:], 0.125)
                    # per-query scale gate: c = 0.25*sigmoid(qp . sw)
                    #   = 2*qp8*(1 + tanh(psc/2)) with psc = (8*sw)^T qp8
                    psc = projp.tile([128, 512], FP32, tag="proj")
                    for hh in range(2):
                        nc.tensor.matmul(
                            psc[32 * hh:32 * hh + 1, :],
                            lhsT=swb_sb[:, hh:hh + 1], rhs=qp8[:],
                            start=True, stop=True,
                            tile_position=(0, 32 * hh))
                    ct = scr.tile([128, 512], BF16_T, tag="ct")
                    nc.scalar.activation(ct[0:33, :], psc[0:33, :], TANH,
                                         scale=0.5)
                    # partition-broadcast via DRAM roundtrip (0-stride DMA)
                    cbuf = pdram.tile([1, 1024], BF16_T, tag="cbuf")
                    for hh in range(2):
                        nc.sync.dma_start(
                            cbuf[0:1, 512 * hh:512 * (hh + 1)],
                            ct[32 * hh:32 * hh + 1, :])
                    bc = scr.tile([128, 512], BF16_T, tag="bc")
                    for hh in range(2):
                        nc.sync.dma_start(
                            bc[64 * hh:64 * (hh + 1), :],
                            cbuf[0:1, 512 * hh:512 * (hh + 1)]
                            .partition_broadcast(64))
                    nc.vector.scalar_tensor_tensor(
                        qhTs[b][:, ssl], bc[:], 1.0, qp8[:], op0=ADD, op1=MULT)
                else:  # v
                    ps = projp.tile([128, 512], FP32, tag="proj")
                    for ttl in range(4):
                        for i in range(N_IC):
                            nc.tensor.matmul(
                                ps[:, 128 * ttl:128 * (ttl + 1)],
                                lhsT=pan[:, PAN * i + 128 * ttl:
                                         PAN * i + 128 * (ttl + 1)],
                                rhs=wv_sb[:, i, :],
                                start=(i == 0), stop=(i == N_IC - 1))
                    nc.vector.tensor_copy(vp_sb[b][p][:], ps[:])
                    if DEBUG:
                        nc.sync.dma_start(dbg["dbg_vp"][b, p], vp_sb[b][p][:])

            def emit_attn_sc(b, sc):
                ssl = slice(S_CHUNK * sc, S_CHUNK * (sc + 1))
                av = avp.tile([128, S_CHUNK], FP32, tag="av")
                den = denp.tile([128, S_CHUNK], FP32, tag="den")
                acc = [None, None]  # two bf16 running sums (even/odd tt)
                for tt in range(N_TT):
                    tsl = slice(128 * tt, 128 * (tt + 1))
                    sps = spsp.tile([128, 2 * S_CHUNK], FP32, tag="sps",
                                    name=f"sps{b}_{sc}_{tt}")
                    for hh in range(2):  # row-tiled K=64 pair
                        rows = slice(64 * hh, 64 * (hh + 1))
                        nc.tensor.matmul(
                            sps[:, S_CHUNK * hh:S_CHUNK * (hh + 1)],
                            lhsT=kpTs[b][rows, tsl],
                            rhs=qhTs[b][rows, ssl],
                            start=True, stop=True)
                    at = attnp.tile([128, 2 * S_CHUNK], BF16_T, tag="at",
                                    name=f"at{b}_{sc}_{tt}", bufs=8)
                    nc.scalar.activation(at[:], sps[:], EXP)
                    for hh in range(2):  # attn @ V, col-tiled pair
                        nc.tensor.matmul(
                            av[64 * hh:64 * (hh + 1), :],
                            lhsT=vp_sb[b][tt // 4][:, 128 * (tt % 4) + 64 * hh:
                                                   128 * (tt % 4) + 64 * (hh + 1)],
                            rhs=at[:, S_CHUNK * hh:S_CHUNK * (hh + 1)],
                            start=(tt == 0), stop=(tt == N_TT - 1),
                            skip_group_check=True)
                    if tt < 2:
                        acc[tt] = at
                    else:
                        nc.vector.tensor_add(acc[tt % 2][:], acc[tt % 2][:],
                                             at[:])
                nc.vector.tensor_add(acc[0][:], acc[0][:], acc[1][:])
                # denominator = ones^T @ (summed exp), col-tiled pair
                for hh in range(2):
                    nc.tensor.matmul(
                        den[32 * hh:32 * hh + 1, :],
                        lhsT=ones_sb[:],
                        rhs=acc[0][:, S_CHUNK * hh:S_CHUNK * (hh + 1)],
                        start=True, stop=True,
                        tile_position=(0, 32 * hh),
                        skip_group_check=True)
                rec = nrm.tile([128, S_CHUNK], FP32, tag="rec")
                nc.vector.reciprocal_approx_fast(rec[:], den[:])
                rst = pdram.tile([1, 1024], FP32, tag="rst")
                for hh in range(2):
                    nc.sync.dma_start(
                        rst[0:1, 512 * hh:512 * (hh + 1)],
                        rec[32 * hh:32 * hh + 1, :])
                bcs = nrm.tile([128, S_CHUNK], FP32, tag="bcs")
                for hh in range(2):
                    nc.sync.dma_start(
                        bcs[64 * hh:64 * (hh + 1), :],
                        rst[0:1, 512 * hh:512 * (hh + 1)]
                        .partition_broadcast(64))
                nrm_t = nrm.tile([128, S_CHUNK], BF16_T, tag="nrmt")
                nc.vector.tensor_mul(nrm_t[:], av[:], bcs[:])
                # stage for the per-batch A2A: slot d = query block 256*d
                nc.sync.dma_start(a2a_in[b][2 * sc, :, :], nrm_t[:, 0:256])
                nc.sync.dma_start(a2a_in[b][2 * sc + 1, :, :], nrm_t[:, 256:512])
                if DEBUG:
                    nc.sync.dma_start(dbg["dbg_nrm"][b, sc], nrm_t[:])

            def emit_a2a(b):
                nc.gpsimd.collective_compute(
                    "AllToAll", mybir.AluOpType.bypass,
                    replica_groups=[list(range(N_CORES))],
                    ins=[a2a_in[b].opt()], outs=[a2a_out[b].opt()])

            def emit_stage2(b, yps_pool=None):
                of = a2a_out[b].rearrange("a b c -> (a b) c")
                oT = []
                for j in range(N_IC):
                    t = s2sb.tile([128, 256], BF16_T, name=f"oT_{j}")
                    nc.sync.dma_start(t[:], of[128 * j:128 * (j + 1), :])
                    oT.append(t)
                    if DEBUG:
                        nc.sync.dma_start(dbg["dbg_oT"][b, j], t[:])
                gt = []
                ypsl = None
                if yps_pool is not None:
                    ypsl = [yps_pool.tile([128, 512], FP32, name=f"yps_{i}")
                            for i in range(4)]
                for jc in range(N_IC):
                    gps = projp.tile([128, 512], FP32, tag="proj")
                    for j2 in range(N_IC):
                        nc.tensor.matmul(
                            gps[:, 0:256],
                            lhsT=wg_sb[:, j2, 128 * jc:128 * (jc + 1)],
                            rhs=oT[j2][:],
                            start=(j2 == 0), stop=(j2 == N_IC - 1))
                    tg = s2sb.tile([128, 256], BF16_T, tag="tg", bufs=2)
                    nc.scalar.activation(tg[:], gps[:, 0:256], TANH,
                                         scale=0.5, bias=gb_sb[:, jc:jc + 1])
                    g = s2sb.tile([128, 256], BF16_T, name=f"gt_{jc}")
                    nc.vector.scalar_tensor_tensor(
                        g[:], tg[:], 1.0, oT[jc][:], op0=ADD, op1=MULT)
                    gt.append(g)
                    if ypsl is not None:  # pipelined accumulation (tail phase)
                        for ct in range(N_IC):
                            # start=True clears has_written for the WHOLE
                            # bank, so only the first group per bank may use
                            # it; the odd-ct group starts on cleared bits
                            # (overwrite-where-clear) instead.
                            nc.tensor.matmul(
                                ypsl[ct // 2][:, 256 * (ct % 2):
                                              256 * (ct % 2) + 256],
                                lhsT=wo_sb[:, jc, 128 * ct:128 * (ct + 1)],
                                rhs=g[:],
                                start=(jc == 0 and ct % 2 == 0),
                                stop=(jc == N_IC - 1),
                                skip_group_check=True)
                for ct in range(N_IC):
                    if ypsl is not None:
                        src = ypsl[ct // 2][:, 256 * (ct % 2):256 * (ct % 2) + 256]
                    else:
                        yps = projp.tile([128, 512], FP32, tag="proj")
                        for jc in range(N_IC):
                            nc.tensor.matmul(
                                yps[:, 0:256],
                                lhsT=wo_sb[:, jc, 128 * ct:128 * (ct + 1)],
                                rhs=gt[jc][:],
                                start=(jc == 0), stop=(jc == N_IC - 1))
                        src = yps[:, 0:256]
                    yo = s2sb.tile([128, 256], BF16_T, tag="yo", bufs=2)
                    nc.vector.tensor_copy(yo[:], src)
                    nc.sync.dma_start(
                        yT[128 * ct:128 * (ct + 1), 256 * b:256 * (b + 1)],
                        yo[:])

            # ---------- drive ----------
            # b0 input panels; k and q first (they gate the exp stream),
            # v may lag (attn@V tolerates it, at-tiles buffer)
            ORDER0 = [("k", 0), ("q", 0), ("k", 1), ("k", 2), ("k", 3),
                      ("q", 1), ("v", 0), ("v", 1), ("v", 2), ("v", 3),
                      ("q", 2), ("q", 3)]
            for kind, p in ORDER0:
                emit_panel(0, kind, p)
            if DEBUG:
                nc.sync.dma_start(dbg["dbg_qhTs"][0], qhTs[0][:])
                nc.sync.dma_start(dbg["dbg_kpTs"][0], kpTs[0][:])

            B1_INTERLEAVE = {
                0: [("k", 0), ("q", 0), ("k", 1)],
                1: [("k", 2), ("k", 3), ("q", 1)],
                2: [("v", 0), ("v", 1), ("v", 2), ("v", 3)],
                3: [("q", 2), ("q", 3)],
            }
            for sc in range(N_SC):
                emit_attn_sc(0, sc)
                for kind, p in B1_INTERLEAVE[sc]:
                    emit_panel(1, kind, p)
                if sc == 2:
                    nc.sync.dma_start(
                        wg_sb[:], wg.rearrange("p (c n) -> p c n", c=N_IC))
                    nc.sync.dma_start(
                        wo_sb[:], wo.rearrange("p (c n) -> p c n", c=N_IC))
            emit_a2a(0)
            if DEBUG:
                nc.sync.dma_start(dbg["dbg_qhTs"][1], qhTs[1][:])
                nc.sync.dma_start(dbg["dbg_kpTs"][1], kpTs[1][:])

            for sc in range(N_SC):
                emit_attn_sc(1, sc)
                if sc == 0:
                    emit_stage2(0)  # overlaps b1 attention
            emit_a2a(1)

            # free attention PSUM (LIFO), open tail accumulators
            for cm in (denp_cm, avp_cm, spsp_cm):
                cm.__exit__(None, None, None)
            yps2_cm = tc.tile_pool(name="yps2", bufs=1, space="PSUM")
            yps2 = yps2_cm.__enter__()
            emit_stage2(1, yps_pool=yps2)

            for cm in (yps2_cm, s2sb_cm, dram_cm, pdram_cm, nrm_cm, scr_cm,
                       attnp_cm, projp_cm, xpan_cm):
                cm.__exit__(None, None, None)

    nc.compile()
    return nc


def _shard_inputs(q, k, v, q_proj_weight, k_proj_weight, v_proj_weight,
                  out_proj_weight, gate_weight, gate_bias, scale_weight):
    def _prearr(w):  # [1024, N] -> [128, 8*N]: row p holds chunks c at (c*N..)
        n = w.shape[1]
        return np.ascontiguousarray(
            w.reshape(8, 128, n).transpose(1, 0, 2).reshape(128, 8 * n)).astype(BF16)

    def _panelize(x):  # [B,S,D] -> [B, 128, N_PAN, N_IC*PAN]
        xt = x.transpose(0, 2, 1)                       # [B, D, S]
        xt = xt.reshape(B, N_IC, 128, N_PAN, PAN)       # (b, c, p, pan, s)
        xt = xt.transpose(0, 2, 3, 1, 4)                # (b, p, pan, c, s)
        return np.ascontiguousarray(
            xt.reshape(B, 128, N_PAN, N_IC * PAN)).astype(BF16)

    gbh = np.ascontiguousarray(
        (0.5 * gate_bias).astype(np.float32).reshape(8, 128).T)  # [128, 8]
    wg_h = _prearr(gate_weight)
    wo_h = _prearr(0.5 * out_proj_weight)
    qTh = _panelize(q)
    kTh = _panelize(k)
    vTh = _panelize(v)
    in_maps = []
    for c in range(N_CORES):
        cols = slice(GCOLS * c, GCOLS * (c + 1))
        swb = np.zeros((GCOLS, 2), np.float32)
        swb[0:64, 0] = 8.0 * scale_weight[2 * c]
        swb[64:128, 1] = 8.0 * scale_weight[2 * c + 1]
        in_maps.append({
            "qT": qTh,
            "kT": kTh,
            "vT": vTh,
            "wq": _prearr(q_proj_weight[:, cols]),
            "wk": _prearr(k_proj_weight[:, cols]),
            "wv": _prearr(v_proj_weight[:, cols]),
            "swb": swb.astype(BF16),
            "wg": wg_h,
            "wo": wo_h,
            "gb": gbh,
        })
    return in_maps


def kernel(**inputs):
    global LAST_RESULTS
    if "nc" not in _CACHE:
        _CACHE["nc"] = _build_nc()
    nc = _CACHE["nc"]
    if TRACE:
        _ensure_ntff_hook()
    in_maps = _shard_inputs(**{k: np.asarray(v) for k, v in inputs.items()})
    res = bass_utils.run_bass_kernel_spmd(
        nc, in_maps, core_ids=list(range(N_CORES)),
        trace=TRACE, trace_kwargs=TRACE_KWARGS,
    )
    LAST_RESULTS = res
    y = np.zeros((B, S, D_MODEL), np.float32)
    for c in range(N_CORES):
        yt = np.asarray(res.results[c]["yT"], np.float32)
        y[0, 256 * c:256 * (c + 1), :] = yt[:, 0:256].T
        y[1, 256 * c:256 * (c + 1), :] = yt[:, 256:512].T
    return y


if __name__ == "__main__":
    rng = np.random.default_rng(0)
    fake = {
        "q": rng.normal(size=(B, S, D_MODEL)).astype(np.float32),
        "k": rng.normal(size=(B, S, D_MODEL)).astype(np.float32),
        "v": rng.normal(size=(B, S, D_MODEL)).astype(np.float32),
        "q_proj_weight": rng.normal(size=(D_MODEL, D_MODEL)).astype(np.float32) * 0.02,
        "k_proj_weight": rng.normal(size=(D_MODEL, D_MODEL)).astype(np.float32) * 0.02,
        "v_proj_weight": rng.normal(size=(D_MODEL, D_MODEL)).astype(np.float32) * 0.02,
        "out_proj_weight": rng.normal(size=(D_MODEL, D_MODEL)).astype(np.float32) * 0.02,
        "gate_weight": rng.normal(size=(D_MODEL, D_MODEL)).astype(np.float32) * 0.02,
        "gate_bias": rng.normal(size=(D_MODEL,)).astype(np.float32) * 0.02,
        "scale_weight": rng.normal(size=(NHEAD, HEAD_DIM)).astype(np.float32) * 0.02,
    }
    out = kernel(**fake)
    print("ran", out.shape, out.dtype)
